# revision 138
# baseline (speedup 1.0000x reference)
"""Multi-head attention (B=4, L=2048, D=1024, H=16, hd=64) on 8 Trainium2 cores.

Sharding: 8-way tensor parallel over heads. Core c owns heads (2c, 2c+1) for
all batches: it projects qkv for its heads (x replicated, w_qkv column-sliced),
runs attention, and computes a partial out-projection with its w_out row-slice.
The host sums the 8 partials (row-parallel unshard).

All matmul operands are fp16 (PSUM accumulation stays fp32): 16-bit rhs
streams through the PE at 1 cycle/column (fp32/fp32r takes 2), and 16-bit
weights enable fast-weight-load. fp16 (e5m10) keeps ~2x the mantissa of the
fp32r baseline's effective rounding, so accuracy stays ~1e-3.

Per-core kernel:
  A) qkvT = W_slice.T @ x computed transposed: lhsT = W tiles, rhs = xT tiles
     -> qT/kT [128 rows = 2 heads x 64 dim, tokens] kept in SBUF (per batch);
     vT is PE-transposed into natural V layout with a ones column appended
     (the ones column makes the PV matmul emit the softmax denominator).
  B) Flash-style attention in S^T layout (keys on partitions), both heads in
     lockstep: the two S matmuls per key tile have K=64 so they land in
     distinct PE row-groups (lhsT base partitions 0/64) and co-execute (~2x).
     One fused exp per key tile covers both heads' scores ([128,1024] ACT op,
     1/8 scale folded in; no max subtraction: scores are bounded ~N(0,1)).
     P^T is the PV matmul rhs with V as weights -> o^T[65, i], row 64 the
     denominator (ones column in V); 1/denom via DVE reciprocal_approx_fast,
     partition-broadcast on the otherwise-idle GpSimd engine, applied by a
     DVE multiply deferred off the PE critical path.
  C) y_partial = oT.T @ w_out_slice, streamed out per 128-token tile.

Scheduling: per j-iteration the next S pair is emitted AHEAD of the current
PV pair so the ACT exp stream (which gates PV) never waits on a PE
round-trip - exp runs gapless at ~1.01us per [128,1024] tile and clocks the
loop. The in-order PE queue is padded with "filler" units (stage A of the
next batch, v transposes, out-projection token tiles) split to <1us each and
paced over the batch's j iterations; an open PSUM accumulation group chains
its second half as the forced next pop so no other PSUM-allocating unit can
interleave. This keeps TensorE duty ~88% and the HAM clock gate warm.
"""
import os
from collections import deque
import numpy as np
from contextlib import ExitStack

B, L, D = 4, 2048, 1024
NH, HD = 16, 64
T = B * L  # 8192 tokens
NCORES = 8
TM = 512  # stage-A token macro-tile
IM = 512  # stage-B query macro-tile


def _build_program():
    import concourse.bacc as bacc
    import concourse.tile as tile
    from concourse import mybir

    F32 = mybir.dt.float32
    F16 = mybir.dt.float16
    EXP = mybir.ActivationFunctionType.Exp

    nc = bacc.Bacc(
        "TRN2", target_bir_lowering=False, debug=False, num_devices=NCORES
    )
    xT_d = nc.dram_tensor("xT", [D, T], F16, kind="ExternalInput")
    wqkv_d = nc.dram_tensor("wqkv", [D, 384], F16, kind="ExternalInput")
    wout_d = nc.dram_tensor("wout", [128, D], F16, kind="ExternalInput")
    ones_d = nc.dram_tensor("ones", [128, 64], F16, kind="ExternalInput")
    ident_d = nc.dram_tensor("ident", [128, 128], F16, kind="ExternalInput")
    y_d = nc.dram_tensor("y", [T, D], F16, kind="ExternalOutput")

    xT_v = xT_d.ap().rearrange("(k p) t -> p k t", p=128)  # [128, 8, T]
    wqkv_v = wqkv_d.ap().rearrange("(k p) c -> p k c", p=128)  # [128, 8, 384]

    NTM = L // TM  # stage-A macro tiles per batch
    NJ = L // 128  # key tiles per batch
    NIM = L // IM  # query macro tiles per batch

    with tile.TileContext(nc) as tc, ExitStack() as ctx:
        consts = ctx.enter_context(tc.tile_pool(name="consts", bufs=1))
        sb_x = ctx.enter_context(tc.tile_pool(name="sb_x", bufs=6))
        sb_qk = ctx.enter_context(tc.tile_pool(name="sb_qk", bufs=2))
        sb_v = ctx.enter_context(tc.tile_pool(name="sb_v", bufs=2))
        sb_vst = ctx.enter_context(tc.tile_pool(name="sb_vst", bufs=5))
        sb_p = ctx.enter_context(tc.tile_pool(name="sb_p", bufs=5))
        sb_o = ctx.enter_context(tc.tile_pool(name="sb_o", bufs=8))
        sb_oT = ctx.enter_context(tc.tile_pool(name="sb_oT", bufs=6))
        sb_y = ctx.enter_context(tc.tile_pool(name="sb_y", bufs=7))
        ps_s = ctx.enter_context(tc.tile_pool(name="ps_s", bufs=2, space="PSUM"))
        ps_po = ctx.enter_context(tc.tile_pool(name="ps_po", bufs=2, space="PSUM"))
        ps_m = ctx.enter_context(tc.tile_pool(name="ps_m", bufs=2, space="PSUM"))

        # DMA issue order matters at kernel start: only the q-column weight
        # chunk and the first x tile gate the first matmuls; everything else
        # streams underneath stage-A compute via a deferred unit.
        wq_t = consts.tile([128, 8, 384], F16, tag="wqkv")
        nc.sync.dma_start(wq_t[:, :, 0:128], wqkv_v[:, :, 0:128])
        ones_t = consts.tile([128, 64], F16, tag="ones")
        nc.sync.dma_start(ones_t[:], ones_d[:])
        ident_t = consts.tile([128, 128], F16, tag="ident")
        nc.sync.dma_start(ident_t[:], ident_d[:])
        wo_t = consts.tile([128, D], F16, tag="wout")

        def deferred_consts():
            for c in range(1, 3):
                nc.sync.dma_start(
                    wq_t[:, :, c * 128 : (c + 1) * 128],
                    wqkv_v[:, :, c * 128 : (c + 1) * 128],
                )

        qk_tiles = {}  # b -> (qT, kT, v_aug)

        def stage_a_units(b):
            """Return emitter closures for batch b's qkv projection."""
            qT_b = sb_qk.tile([128, L], F16, tag="qT")
            kT_b = sb_qk.tile([128, L], F16, tag="kT")
            v_b = sb_v.tile([128, NJ, 2, 65], F16, tag="v")
            qk_tiles[b] = (qT_b, kT_b, v_b)
            xt_tiles = {}
            vst_tiles = {}

            def ones_col():
                nc.vector.tensor_copy(
                    v_b[:, :, :, 64:65],
                    ones_t[:, 0 : 2 * NJ].rearrange(
                        "p (j h o) -> p j h o", h=2, o=1
                    ),
                )

            psA_open = {}

            def xt_load(tm):
                xt = sb_x.tile([128, 8, TM], F16, tag="xt")
                t0 = b * L + tm * TM
                # two chunks so the first k-matmuls gate on half the bytes
                nc.sync.dma_start(xt[:, 0:4, :], xT_v[:, 0:4, t0 : t0 + TM])
                nc.sync.dma_start(xt[:, 4:8, :], xT_v[:, 4:8, t0 : t0 + TM])
                xt_tiles[tm] = xt

            def col_group_a(tm, c):
                xt = xt_tiles[tm]
                psA = ps_m.tile([128, TM], mybir.dt.float32, tag="m")
                psA_open[(tm, c)] = psA
                for k in range(4):
                    nc.tensor.matmul(
                        psA[:],
                        wq_t[:, k, c * 128 : (c + 1) * 128],
                        xt[:, k, :],
                        start=(k == 0),
                        stop=False,
                    )

            def col_group_b(tm, c):
                xt = xt_tiles[tm]
                psA = psA_open.pop((tm, c))
                for k in range(4, 8):
                    nc.tensor.matmul(
                        psA[:],
                        wq_t[:, k, c * 128 : (c + 1) * 128],
                        xt[:, k, :],
                        start=False,
                        stop=(k == 7),
                    )
                if c == 0:
                    nc.vector.tensor_copy(qT_b[:, tm * TM : (tm + 1) * TM], psA[:])
                elif c == 1:
                    nc.vector.tensor_copy(kT_b[:, tm * TM : (tm + 1) * TM], psA[:])
                else:
                    vst = sb_vst.tile([128, TM], F16, tag="vst")
                    nc.vector.tensor_copy(vst[:], psA[:])
                    vst_tiles[tm] = vst

            def transposes(tm, half):
                vst = vst_tiles[tm]
                for tb in range(half * 2, half * 2 + 2):
                    jt = tm * (TM // 128) + tb
                    ptr = ps_m.tile([128, 128], F16, tag="m")
                    nc.tensor.transpose(
                        ptr[:], vst[:, tb * 128 : (tb + 1) * 128], ident_t[:]
                    )
                    nc.vector.tensor_copy(
                        v_b[:, jt, :, 0:64],
                        ptr[:].rearrange("p (h d) -> p h d", h=2),
                    )

            # units are (cost, fn, follow): `follow` MUST be the next
            # ps_m-allocating pop (an open accumulation group may not be
            # interleaved with another ps_m allocation)
            units = [
                (0.1, ones_col, None),
                (0.01, lambda: xt_load(0), None),
                (0.01, lambda: xt_load(1), None),
            ]
            for tm in range(NTM):
                for c in range(3):
                    units.append(
                        (
                            0.5,
                            lambda tm=tm, c=c: col_group_a(tm, c),
                            lambda tm=tm, c=c: col_group_b(tm, c),
                        )
                    )
                    if c == 0 and tm + 2 < NTM:
                        # prefetch two x macro-tiles ahead of the consumer
                        units.append(
                            (0.01, lambda tm=tm: xt_load(tm + 2), None)
                        )
                for half in range(2):
                    units.append(
                        (0.3, lambda tm=tm, half=half: transposes(tm, half), None)
                    )
            return units

        filler = deque()  # batch-deadline units (stage A, norm tails)

        pending_norm = []
        pace = {"credit": 0.0, "iters_left": 1}

        def pop_filler():
            """Cost-weighted pacing: spread queued filler evenly over the
            batch's j iterations instead of draining it front-loaded. A unit
            with a `follow` closure opens a PSUM accumulation group: its
            follow runs first at the next call, before any other unit."""
            done = 0.0
            if pace.get("forced") is not None:
                fn = pace.pop("forced")
                fn()
                done += 0.5
            total = sum(c for c, _, _, _ in filler)
            iters_left = pace.get("iters_left", 1)
            rate = total / max(iters_left, 1)
            pace["credit"] += rate
            while filler and pace["credit"] >= filler[0][0] * 0.5:
                c, fn, follow, _d = filler.popleft()
                pace["credit"] -= c
                done += c
                fn()
                if follow is not None:
                    pace["forced"] = follow
                    break
            pace["iters_left"] = max(iters_left - 1, 1)
            return done

        def stage_b(b):
            """Both heads in lockstep: the two S matmuls per j land in
            distinct PE row-groups (lhsT base partitions 0/64, K=64 each) so
            they co-execute; one fused exp covers both heads' scores."""
            qT_b, kT_b, v_b = qk_tiles[b]
            # last batch: drain slower so leftover proj units pad the PE
            # through the end-of-kernel norm chain (deferral depth is safe:
            # single FIFO + sb_oT bufs=6 / sb_o bufs=8 give ~4 ims of slack)
            pace["iters_left"] = int(NIM * NJ * (1.25 if b == B - 1 else 0.8))
            for im in range(NIM):
                oT_b = sb_oT.tile([128, IM], F16, tag="oT")
                po = [
                    ps_po.tile([65, IM], mybir.dt.float32, tag="po", name=f"po{_h}")
                    for _h in range(2)
                ]
                s_tiles = {}

                def s_pair(j):
                    s2 = ps_s.tile([128, 2 * IM], mybir.dt.float32, tag="s")
                    for h in range(2):
                        hb = h * 64
                        nc.tensor.matmul(
                            s2[:, h * IM : (h + 1) * IM],
                            kT_b[hb : hb + 64, j * 128 : (j + 1) * 128],
                            qT_b[hb : hb + 64, im * IM : (im + 1) * IM],
                            start=True,
                            stop=True,
                        )
                    s_tiles[j] = s2

                p_prev = None
                with tc.high_priority(48):
                    s_pair(0)
                for j in range(NJ):
                    # the next S pair goes on the PE queue ahead of this
                    # round's PV pair: it has no exp dependency, so the ACT
                    # exp stream (which gates PV) stays gapless. The small
                    # priority nudge lets the scheduler pull it ahead of
                    # ~one j-iteration of filler matmuls.
                    if j + 1 < NJ:
                        with tc.high_priority(48):
                            s_pair(j + 1)
                    if j == 0 and pending_norm:
                        for fn in pending_norm:
                            fn()
                        pending_norm.clear()
                    p_t = sb_p.tile([128, 2 * IM], F16, tag="p")
                    nc.scalar.activation(p_t[:], s_tiles.pop(j)[:], EXP,
                                         scale=0.125)
                    if p_prev is not None:
                        jp, pp = p_prev
                        for h in range(2):
                            nc.tensor.matmul(
                                po[h][:],
                                v_b[:, jp, h, :],
                                pp[:, h * IM : (h + 1) * IM],
                                start=(jp == 0),
                                stop=False,
                            )
                    p_prev = (j, p_t)
                    pop_filler()
                jp, pp = p_prev
                for h in range(2):
                    nc.tensor.matmul(
                        po[h][:],
                        v_b[:, jp, h, :],
                        pp[:, h * IM : (h + 1) * IM],
                        start=False,
                        stop=True,
                    )

                def norm_d(po=po, oT_b=oT_b, im=im, b=b):
                    # fast part at flush: DVE evacuation copies free the po
                    # PSUM slots; both heads' denominator rows get packed
                    # into one 2-lane tile so one approx-reciprocal (660ns,
                    # ~18 correct bits - plenty for fp16 storage) covers
                    # them. The partition broadcast of 1/denom runs on the
                    # otherwise-idle GpSimd engine, and the oT multiply is
                    # deferred into the filler so the PE stream never waits.
                    o_ev = [
                        sb_o.tile([65, IM], mybir.dt.float32, tag="oe",
                                  name=f"oe{_h}")
                        for _h in range(2)
                    ]
                    den = [
                        sb_o.tile([1, IM], mybir.dt.float32, tag="den",
                                  name=f"dn{_h}")
                        for _h in range(2)
                    ]
                    rec = [
                        sb_o.tile([1, IM], mybir.dt.float32, tag="rec",
                                  name=f"rc{_h}")
                        for _h in range(2)
                    ]
                    bc = [
                        sb_o.tile([64, IM], mybir.dt.float32, tag="bc",
                                  name=f"bc{_h}")
                        for _h in range(2)
                    ]
                    # boosted: these release the po PSUM slots that gate
                    # the next im's first PV accumulation
                    with tc.high_priority(48):
                        for h in range(2):
                            nc.vector.tensor_copy(
                                o_ev[h][0:64, :], po[h][0:64, :]
                            )
                            nc.vector.tensor_copy(den[h][:], po[h][64:65, :])
                            nc.vector.reciprocal_approx_fast(
                                out=rec[h][:], in_=den[h][:]
                            )
                            nc.gpsimd.partition_broadcast(bc[h][:], rec[h][:])

                    def tail():
                        with nc.allow_low_precision(reason="fp16"):
                            for h in range(2):
                                hb = h * 64
                                nc.vector.tensor_mul(
                                    oT_b[hb : hb + 64, :],
                                    o_ev[h][0:64, :],
                                    bc[h][:],
                                )

                    filler.append((0.2, tail, None, b + 1))
                    for ts in range(IM // 128):
                        filler.append(
                            (0.35, lambda ts=ts: proj_ts(ts, b, im, oT_b),
                             None, b + 1)
                        )

                if b == B - 1:
                    # the last batch has no following stage A to fill gaps;
                    # start the DVE norm chain immediately so its
                    # tails/projs enqueue early
                    norm_d()
                else:
                    pending_norm.append(norm_d)

                def proj_ts(ts, b=b, im=im, oT_b=oT_b):
                    # in the end-of-kernel tail the exp stream is done, so
                    # ScalarE is idle - route the PSUM evacuations there to
                    # halve the psC slot round-trip that gates the PE
                    evac_act = b == B - 1 and im >= NIM - 2
                    y_t = sb_y.tile([128, D], F16, tag="y")
                    for nh in range(2):
                        psC = ps_m.tile([128, 512], mybir.dt.float32, tag="m")
                        nc.tensor.matmul(
                            psC[:],
                            oT_b[:, ts * 128 : (ts + 1) * 128],
                            wo_t[:, nh * 512 : (nh + 1) * 512],
                            start=True,
                            stop=True,
                        )
                        if evac_act and nh == 1:
                            nc.scalar.activation(
                                y_t[:, nh * 512 : (nh + 1) * 512],
                                psC[:],
                                mybir.ActivationFunctionType.Copy,
                            )
                        else:
                            nc.vector.tensor_copy(
                                y_t[:, nh * 512 : (nh + 1) * 512], psC[:]
                            )
                    t0 = b * L + im * IM + ts * 128
                    nc.sync.dma_start(y_d[t0 : t0 + 128, :], y_t[:])

        # HAM warm-up: ~30 dummy matmuls on the identity tile (resident
        # within ~200ns) run during the initial x/w DMA wait, so the PE
        # clock gate is already at 8/8 when the first real matmul issues.
        warm_ps = ps_m.tile([128, 128], mybir.dt.float32, tag="m")
        for w in range(30):
            nc.tensor.matmul(
                warm_ps[:], ident_t[:], ident_t[:],
                start=(w == 0), stop=(w == 29),
            )
        # dummy exp pulls the ~1.3us ACT_TABLE_LOAD off the first real
        # exp's critical path (ACT idles through all of batch-0 stage A);
        # the output tile lives in the stable consts pool so the cyclic
        # pools' layout is untouched
        actwarm_t = consts.tile([128, 64], F16, tag="actwarm")
        nc.scalar.activation(actwarm_t[:], ones_t[:], EXP, scale=0.125)

        # batch 0 stage A runs eagerly; later batches go through the filler
        NB = int(os.environ.get("ATTN_KERNEL_BATCHES", str(B)))
        for idx, (_c, u, follow) in enumerate(stage_a_units(0)):
            u()
            if follow is not None:
                follow()
            if idx == 1:
                # first x tile + q-weights are in flight; now queue the rest
                deferred_consts()
        # wout is first needed ~10 j-iterations into stage B - keep its
        # bytes out of the bandwidth-critical start window
        nc.sync.dma_start(wo_t[:], wout_d[:])
        for b in range(NB):
            if b + 1 < NB:
                for c, fn, follow in stage_a_units(b + 1):
                    if c == 0.01:
                        # all four xt_loads issue their DMAs at batch start:
                        # mid-batch issue crawls behind y-output traffic
                        # (the old 5-6us tm2/3 stalls); the deeper sb_y pool
                        # absorbs the burst's added y latency
                        fn()
                    else:
                        filler.append((c, fn, follow, b + 1))
            stage_b(b)
        for fn in pending_norm:
            fn()
        pending_norm.clear()
        if pace.get("forced") is not None:
            pace.pop("forced")()
        while filler:
            _c, fn, follow, _d = filler.popleft()
            fn()
            if follow is not None:
                follow()

    # Exp and Ln both live in the natural_log_exp_and_others ACT table set;
    # hide the single-function sets so the chooser can't thrash between them
    # (each ACT_TABLE_LOAD swap costs ~2.7us and stalls the exp stream).
    import concourse.bacc as bacc_mod

    orig_gat = bacc_mod.get_activation_tables

    def _combined_tables(arch):
        # keep positions intact (act_func_set_id indexes this list); just
        # empty the sets we don't want so the chooser can't pick them
        tabs = dict(orig_gat(arch))
        for bad in ("exp_and_others", "natural_log", "exp_and_friends"):
            if bad in tabs:
                tabs[bad] = type(tabs[bad])()
        return tabs

    if os.environ.get("ATTN_KERNEL_TABLES", "1") == "1":
        bacc_mod.get_activation_tables = _combined_tables
    try:
        nc.compile()
    finally:
        bacc_mod.get_activation_tables = orig_gat
    return nc


_PROGRAM = None
_LAST_EXEC_NS = None
_LAST_RESULT = None


def _get_program():
    global _PROGRAM
    if _PROGRAM is None:
        _PROGRAM = _build_program()
    return _PROGRAM


def kernel(x, mask, w_qkv, w_out):
    x = np.asarray(x)
    mask = np.asarray(mask)
    w_qkv = np.asarray(w_qkv)
    w_out = np.asarray(w_out)
    if not mask.all():
        return _masked_fallback(x, mask, w_qkv, w_out)

    from concourse.bass_utils import run_bass_kernel_spmd

    xT = np.ascontiguousarray(x.reshape(T, D).T).astype(np.float16)
    w4 = w_qkv.reshape(D, 3, NH, HD)
    ones = np.ones((128, 64), dtype=np.float16)
    ident = np.eye(128, dtype=np.float16)
    in_maps = []
    for c in range(NCORES):
        hsel = [2 * c, 2 * c + 1]
        wc = w4[:, :, hsel, :].reshape(D, 384).astype(np.float16)
        woc = w_out[2 * c * HD : (2 * c + 2) * HD, :].astype(np.float16)
        in_maps.append(
            {"xT": xT, "wqkv": wc, "wout": woc, "ones": ones, "ident": ident}
        )

    nc = _get_program()
    trace = os.environ.get("BASS_KERNEL_TRACE") == "1"
    res = run_bass_kernel_spmd(nc, in_maps, list(range(NCORES)), trace=trace)
    global _LAST_EXEC_NS, _LAST_RESULT
    _LAST_RESULT = res
    _LAST_EXEC_NS = getattr(res, "exec_time_ns", None)
    y = res.results[0]["y"].astype(np.float64)
    for c in range(1, NCORES):
        y += res.results[c]["y"]
    return y.astype(np.float32).reshape(B, L, D)


def _masked_fallback(x, mask, w_qkv, w_out):
    """Reference path for non-all-true masks (never hit for the spec inputs)."""
    b, l, d = x.shape
    scale = HD ** -0.5
    qkv = x.reshape(b * l, d) @ w_qkv
    qkv = qkv.reshape(b, l, 3, NH, HD).transpose(2, 0, 3, 1, 4)
    q, k, v = qkv[0], qkv[1], qkv[2]
    attn = np.einsum("bhnd,bhmd->bhnm", q, k) * scale
    attn = np.where(mask[:, None, :, :], attn, -np.inf)
    attn = attn - attn.max(axis=-1, keepdims=True)
    np.exp(attn, out=attn)
    attn /= attn.sum(axis=-1, keepdims=True)
    out = np.einsum("bhnm,bhmd->bhnd", attn, v)
    out = out.transpose(0, 2, 1, 3).reshape(b, l, d)
    return (out @ w_out).astype(np.float32)


# revision 139
# speedup vs baseline: 1.0027x; 1.0027x over previous
"""Multi-head attention (B=4, L=2048, D=1024, H=16, hd=64) on 8 Trainium2 cores.

Sharding: 8-way tensor parallel over heads. Core c owns heads (2c, 2c+1) for
all batches: it projects qkv for its heads (x replicated, w_qkv column-sliced),
runs attention, and computes a partial out-projection with its w_out row-slice.
The host sums the 8 partials (row-parallel unshard).

All matmul operands are fp16 (PSUM accumulation stays fp32): 16-bit rhs
streams through the PE at 1 cycle/column (fp32/fp32r takes 2), and 16-bit
weights enable fast-weight-load. fp16 (e5m10) keeps ~2x the mantissa of the
fp32r baseline's effective rounding, so accuracy stays ~1e-3.

Per-core kernel:
  A) qkvT = W_slice.T @ x computed transposed: lhsT = W tiles, rhs = xT tiles
     -> qT/kT [128 rows = 2 heads x 64 dim, tokens] kept in SBUF (per batch);
     vT is PE-transposed into natural V layout with a ones column appended
     (the ones column makes the PV matmul emit the softmax denominator).
  B) Flash-style attention in S^T layout (keys on partitions), both heads in
     lockstep: the two S matmuls per key tile have K=64 so they land in
     distinct PE row-groups (lhsT base partitions 0/64) and co-execute (~2x).
     One fused exp per key tile covers both heads' scores ([128,1024] ACT op,
     1/8 scale folded in; no max subtraction: scores are bounded ~N(0,1)).
     P^T is the PV matmul rhs with V as weights -> o^T[65, i], row 64 the
     denominator (ones column in V); 1/denom via DVE reciprocal_approx_fast,
     partition-broadcast on the otherwise-idle GpSimd engine, applied by a
     DVE multiply deferred off the PE critical path.
  C) y_partial = oT.T @ w_out_slice, streamed out per 128-token tile.

Scheduling: per j-iteration the next S pair is emitted AHEAD of the current
PV pair so the ACT exp stream (which gates PV) never waits on a PE
round-trip - exp runs gapless at ~1.01us per [128,1024] tile and clocks the
loop. The in-order PE queue is padded with "filler" units (stage A of the
next batch, v transposes, out-projection token tiles) split to <1us each and
paced over the batch's j iterations; an open PSUM accumulation group chains
its second half as the forced next pop so no other PSUM-allocating unit can
interleave. This keeps TensorE duty ~88% and the HAM clock gate warm.
"""
import os
from collections import deque
import numpy as np
from contextlib import ExitStack

B, L, D = 4, 2048, 1024
NH, HD = 16, 64
T = B * L  # 8192 tokens
NCORES = 8
TM = 512  # stage-A token macro-tile
IM = 512  # stage-B query macro-tile


def _build_program():
    import concourse.bacc as bacc
    import concourse.tile as tile
    from concourse import mybir

    F32 = mybir.dt.float32
    F16 = mybir.dt.float16
    EXP = mybir.ActivationFunctionType.Exp

    nc = bacc.Bacc(
        "TRN2", target_bir_lowering=False, debug=False, num_devices=NCORES
    )
    xT_d = nc.dram_tensor("xT", [D, T], F16, kind="ExternalInput")
    wqkv_d = nc.dram_tensor("wqkv", [D, 384], F16, kind="ExternalInput")
    wout_d = nc.dram_tensor("wout", [128, D], F16, kind="ExternalInput")
    ones_d = nc.dram_tensor("ones", [128, 64], F16, kind="ExternalInput")
    ident_d = nc.dram_tensor("ident", [128, 128], F16, kind="ExternalInput")
    y_d = nc.dram_tensor("y", [T, D], F16, kind="ExternalOutput")

    xT_v = xT_d.ap().rearrange("(k p) t -> p k t", p=128)  # [128, 8, T]
    wqkv_v = wqkv_d.ap().rearrange("(k p) c -> p k c", p=128)  # [128, 8, 384]

    NTM = L // TM  # stage-A macro tiles per batch
    NJ = L // 128  # key tiles per batch
    NIM = L // IM  # query macro tiles per batch

    with tile.TileContext(nc) as tc, ExitStack() as ctx:
        consts = ctx.enter_context(tc.tile_pool(name="consts", bufs=1))
        sb_x = ctx.enter_context(tc.tile_pool(name="sb_x", bufs=6))
        sb_qk = ctx.enter_context(tc.tile_pool(name="sb_qk", bufs=2))
        sb_v = ctx.enter_context(tc.tile_pool(name="sb_v", bufs=2))
        sb_vst = ctx.enter_context(tc.tile_pool(name="sb_vst", bufs=5))
        sb_p = ctx.enter_context(tc.tile_pool(name="sb_p", bufs=5))
        sb_o = ctx.enter_context(tc.tile_pool(name="sb_o", bufs=8))
        sb_oT = ctx.enter_context(tc.tile_pool(name="sb_oT", bufs=6))
        sb_y = ctx.enter_context(tc.tile_pool(name="sb_y", bufs=8))
        ps_s = ctx.enter_context(tc.tile_pool(name="ps_s", bufs=2, space="PSUM"))
        ps_po = ctx.enter_context(tc.tile_pool(name="ps_po", bufs=2, space="PSUM"))
        ps_m = ctx.enter_context(tc.tile_pool(name="ps_m", bufs=2, space="PSUM"))

        # DMA issue order matters at kernel start: only the q-column weight
        # chunk and the first x tile gate the first matmuls; everything else
        # streams underneath stage-A compute via a deferred unit.
        wq_t = consts.tile([128, 8, 384], F16, tag="wqkv")
        nc.sync.dma_start(wq_t[:, :, 0:128], wqkv_v[:, :, 0:128])
        ones_t = consts.tile([128, 64], F16, tag="ones")
        nc.sync.dma_start(ones_t[:], ones_d[:])
        ident_t = consts.tile([128, 128], F16, tag="ident")
        nc.sync.dma_start(ident_t[:], ident_d[:])
        wo_t = consts.tile([128, D], F16, tag="wout")

        def deferred_consts():
            for c in range(1, 3):
                nc.sync.dma_start(
                    wq_t[:, :, c * 128 : (c + 1) * 128],
                    wqkv_v[:, :, c * 128 : (c + 1) * 128],
                )

        qk_tiles = {}  # b -> (qT, kT, v_aug)

        def stage_a_units(b):
            """Return emitter closures for batch b's qkv projection."""
            qT_b = sb_qk.tile([128, L], F16, tag="qT")
            kT_b = sb_qk.tile([128, L], F16, tag="kT")
            v_b = sb_v.tile([128, NJ, 2, 65], F16, tag="v")
            qk_tiles[b] = (qT_b, kT_b, v_b)
            xt_tiles = {}
            vst_tiles = {}

            def ones_col():
                nc.vector.tensor_copy(
                    v_b[:, :, :, 64:65],
                    ones_t[:, 0 : 2 * NJ].rearrange(
                        "p (j h o) -> p j h o", h=2, o=1
                    ),
                )

            psA_open = {}

            def xt_load(tm):
                xt = sb_x.tile([128, 8, TM], F16, tag="xt")
                t0 = b * L + tm * TM
                # two chunks so the first k-matmuls gate on half the bytes
                nc.sync.dma_start(xt[:, 0:4, :], xT_v[:, 0:4, t0 : t0 + TM])
                nc.sync.dma_start(xt[:, 4:8, :], xT_v[:, 4:8, t0 : t0 + TM])
                xt_tiles[tm] = xt

            def col_group_a(tm, c):
                xt = xt_tiles[tm]
                psA = ps_m.tile([128, TM], mybir.dt.float32, tag="m")
                psA_open[(tm, c)] = psA
                for k in range(4):
                    nc.tensor.matmul(
                        psA[:],
                        wq_t[:, k, c * 128 : (c + 1) * 128],
                        xt[:, k, :],
                        start=(k == 0),
                        stop=False,
                    )

            def col_group_b(tm, c):
                xt = xt_tiles[tm]
                psA = psA_open.pop((tm, c))
                for k in range(4, 8):
                    nc.tensor.matmul(
                        psA[:],
                        wq_t[:, k, c * 128 : (c + 1) * 128],
                        xt[:, k, :],
                        start=False,
                        stop=(k == 7),
                    )
                if c == 0:
                    nc.vector.tensor_copy(qT_b[:, tm * TM : (tm + 1) * TM], psA[:])
                elif c == 1:
                    nc.vector.tensor_copy(kT_b[:, tm * TM : (tm + 1) * TM], psA[:])
                else:
                    vst = sb_vst.tile([128, TM], F16, tag="vst")
                    nc.vector.tensor_copy(vst[:], psA[:])
                    vst_tiles[tm] = vst

            def transposes(tm, half):
                vst = vst_tiles[tm]
                for tb in range(half * 2, half * 2 + 2):
                    jt = tm * (TM // 128) + tb
                    ptr = ps_m.tile([128, 128], F16, tag="m")
                    nc.tensor.transpose(
                        ptr[:], vst[:, tb * 128 : (tb + 1) * 128], ident_t[:]
                    )
                    nc.vector.tensor_copy(
                        v_b[:, jt, :, 0:64],
                        ptr[:].rearrange("p (h d) -> p h d", h=2),
                    )

            # units are (cost, fn, follow): `follow` MUST be the next
            # ps_m-allocating pop (an open accumulation group may not be
            # interleaved with another ps_m allocation)
            units = [
                (0.1, ones_col, None),
                (0.01, lambda: xt_load(0), None),
                (0.01, lambda: xt_load(1), None),
            ]
            for tm in range(NTM):
                for c in range(3):
                    units.append(
                        (
                            0.5,
                            lambda tm=tm, c=c: col_group_a(tm, c),
                            lambda tm=tm, c=c: col_group_b(tm, c),
                        )
                    )
                    if c == 0 and tm + 2 < NTM:
                        # prefetch two x macro-tiles ahead of the consumer
                        units.append(
                            (0.01, lambda tm=tm: xt_load(tm + 2), None)
                        )
                for half in range(2):
                    units.append(
                        (0.3, lambda tm=tm, half=half: transposes(tm, half), None)
                    )
            return units

        filler = deque()  # batch-deadline units (stage A, norm tails)

        pending_norm = []
        pace = {"credit": 0.0, "iters_left": 1}

        def pop_filler():
            """Cost-weighted pacing: spread queued filler evenly over the
            batch's j iterations instead of draining it front-loaded. A unit
            with a `follow` closure opens a PSUM accumulation group: its
            follow runs first at the next call, before any other unit."""
            done = 0.0
            if pace.get("forced") is not None:
                fn = pace.pop("forced")
                fn()
                done += 0.5
            total = sum(c for c, _, _, _ in filler)
            iters_left = pace.get("iters_left", 1)
            rate = total / max(iters_left, 1)
            pace["credit"] += rate
            while filler and pace["credit"] >= filler[0][0] * 0.5:
                c, fn, follow, _d = filler.popleft()
                pace["credit"] -= c
                done += c
                fn()
                if follow is not None:
                    pace["forced"] = follow
                    break
            pace["iters_left"] = max(iters_left - 1, 1)
            return done

        def stage_b(b):
            """Both heads in lockstep: the two S matmuls per j land in
            distinct PE row-groups (lhsT base partitions 0/64, K=64 each) so
            they co-execute; one fused exp covers both heads' scores."""
            qT_b, kT_b, v_b = qk_tiles[b]
            # last batch: drain slower so leftover proj units pad the PE
            # through the end-of-kernel norm chain (deferral depth is safe:
            # single FIFO + sb_oT bufs=6 / sb_o bufs=8 give ~4 ims of slack)
            pace["iters_left"] = int(NIM * NJ * (1.25 if b == B - 1 else 0.8))
            for im in range(NIM):
                oT_b = sb_oT.tile([128, IM], F16, tag="oT")
                po = [
                    ps_po.tile([65, IM], mybir.dt.float32, tag="po", name=f"po{_h}")
                    for _h in range(2)
                ]
                s_tiles = {}

                def s_pair(j):
                    s2 = ps_s.tile([128, 2 * IM], mybir.dt.float32, tag="s")
                    for h in range(2):
                        hb = h * 64
                        nc.tensor.matmul(
                            s2[:, h * IM : (h + 1) * IM],
                            kT_b[hb : hb + 64, j * 128 : (j + 1) * 128],
                            qT_b[hb : hb + 64, im * IM : (im + 1) * IM],
                            start=True,
                            stop=True,
                        )
                    s_tiles[j] = s2

                p_prev = None
                with tc.high_priority(48):
                    s_pair(0)
                for j in range(NJ):
                    # the next S pair goes on the PE queue ahead of this
                    # round's PV pair: it has no exp dependency, so the ACT
                    # exp stream (which gates PV) stays gapless. The small
                    # priority nudge lets the scheduler pull it ahead of
                    # ~one j-iteration of filler matmuls.
                    if j + 1 < NJ:
                        with tc.high_priority(48):
                            s_pair(j + 1)
                    if j == 0 and pending_norm:
                        for fn in pending_norm:
                            fn()
                        pending_norm.clear()
                    p_t = sb_p.tile([128, 2 * IM], F16, tag="p")
                    nc.scalar.activation(p_t[:], s_tiles.pop(j)[:], EXP,
                                         scale=0.125)
                    if p_prev is not None:
                        jp, pp = p_prev
                        for h in range(2):
                            nc.tensor.matmul(
                                po[h][:],
                                v_b[:, jp, h, :],
                                pp[:, h * IM : (h + 1) * IM],
                                start=(jp == 0),
                                stop=False,
                            )
                    p_prev = (j, p_t)
                    pop_filler()
                jp, pp = p_prev
                for h in range(2):
                    nc.tensor.matmul(
                        po[h][:],
                        v_b[:, jp, h, :],
                        pp[:, h * IM : (h + 1) * IM],
                        start=False,
                        stop=True,
                    )

                def norm_d(po=po, oT_b=oT_b, im=im, b=b):
                    # fast part at flush: DVE evacuation copies free the po
                    # PSUM slots; both heads' denominator rows get packed
                    # into one 2-lane tile so one approx-reciprocal (660ns,
                    # ~18 correct bits - plenty for fp16 storage) covers
                    # them. The partition broadcast of 1/denom runs on the
                    # otherwise-idle GpSimd engine, and the oT multiply is
                    # deferred into the filler so the PE stream never waits.
                    o_ev = [
                        sb_o.tile([65, IM], mybir.dt.float32, tag="oe",
                                  name=f"oe{_h}")
                        for _h in range(2)
                    ]
                    den = [
                        sb_o.tile([1, IM], mybir.dt.float32, tag="den",
                                  name=f"dn{_h}")
                        for _h in range(2)
                    ]
                    rec = [
                        sb_o.tile([1, IM], mybir.dt.float32, tag="rec",
                                  name=f"rc{_h}")
                        for _h in range(2)
                    ]
                    bc = [
                        sb_o.tile([64, IM], mybir.dt.float32, tag="bc",
                                  name=f"bc{_h}")
                        for _h in range(2)
                    ]
                    # boosted: these release the po PSUM slots that gate
                    # the next im's first PV accumulation
                    with tc.high_priority(48):
                        for h in range(2):
                            nc.vector.tensor_copy(
                                o_ev[h][0:64, :], po[h][0:64, :]
                            )
                            nc.vector.tensor_copy(den[h][:], po[h][64:65, :])
                            nc.vector.reciprocal_approx_fast(
                                out=rec[h][:], in_=den[h][:]
                            )
                            nc.gpsimd.partition_broadcast(bc[h][:], rec[h][:])

                    def tail():
                        with nc.allow_low_precision(reason="fp16"):
                            for h in range(2):
                                hb = h * 64
                                nc.vector.tensor_mul(
                                    oT_b[hb : hb + 64, :],
                                    o_ev[h][0:64, :],
                                    bc[h][:],
                                )

                    filler.append((0.2, tail, None, b + 1))
                    for ts in range(IM // 128):
                        filler.append(
                            (0.35, lambda ts=ts: proj_ts(ts, b, im, oT_b),
                             None, b + 1)
                        )

                if b == B - 1:
                    # the last batch has no following stage A to fill gaps;
                    # start the DVE norm chain immediately so its
                    # tails/projs enqueue early
                    norm_d()
                else:
                    pending_norm.append(norm_d)

                def proj_ts(ts, b=b, im=im, oT_b=oT_b):
                    # in the end-of-kernel tail the exp stream is done, so
                    # ScalarE is idle - route the PSUM evacuations there to
                    # halve the psC slot round-trip that gates the PE
                    evac_act = b == B - 1 and im >= NIM - 2
                    y_t = sb_y.tile([128, D], F16, tag="y")
                    for nh in range(2):
                        psC = ps_m.tile([128, 512], mybir.dt.float32, tag="m")
                        nc.tensor.matmul(
                            psC[:],
                            oT_b[:, ts * 128 : (ts + 1) * 128],
                            wo_t[:, nh * 512 : (nh + 1) * 512],
                            start=True,
                            stop=True,
                        )
                        if evac_act and nh == 1:
                            nc.scalar.activation(
                                y_t[:, nh * 512 : (nh + 1) * 512],
                                psC[:],
                                mybir.ActivationFunctionType.Copy,
                            )
                        else:
                            nc.vector.tensor_copy(
                                y_t[:, nh * 512 : (nh + 1) * 512], psC[:]
                            )
                    t0 = b * L + im * IM + ts * 128
                    nc.sync.dma_start(y_d[t0 : t0 + 128, :], y_t[:])

        # HAM warm-up: ~30 dummy matmuls on the identity tile (resident
        # within ~200ns) run during the initial x/w DMA wait, so the PE
        # clock gate is already at 8/8 when the first real matmul issues.
        warm_ps = ps_m.tile([128, 128], mybir.dt.float32, tag="m")
        for w in range(30):
            nc.tensor.matmul(
                warm_ps[:], ident_t[:], ident_t[:],
                start=(w == 0), stop=(w == 29),
            )
        # dummy exp pulls the ~1.3us ACT_TABLE_LOAD off the first real
        # exp's critical path (ACT idles through all of batch-0 stage A);
        # the output tile lives in the stable consts pool so the cyclic
        # pools' layout is untouched
        actwarm_t = consts.tile([128, 64], F16, tag="actwarm")
        nc.scalar.activation(actwarm_t[:], ones_t[:], EXP, scale=0.125)

        # batch 0 stage A runs eagerly; later batches go through the filler
        NB = int(os.environ.get("ATTN_KERNEL_BATCHES", str(B)))
        for idx, (_c, u, follow) in enumerate(stage_a_units(0)):
            u()
            if follow is not None:
                follow()
            if idx == 1:
                # first x tile + q-weights are in flight; now queue the rest
                deferred_consts()
        # wout is first needed ~10 j-iterations into stage B - keep its
        # bytes out of the bandwidth-critical start window
        nc.sync.dma_start(wo_t[:], wout_d[:])
        for b in range(NB):
            if b + 1 < NB:
                for c, fn, follow in stage_a_units(b + 1):
                    if c == 0.01:
                        # all four xt_loads issue their DMAs at batch start:
                        # mid-batch issue crawls behind y-output traffic
                        # (the old 5-6us tm2/3 stalls); the deeper sb_y pool
                        # absorbs the burst's added y latency
                        fn()
                    else:
                        filler.append((c, fn, follow, b + 1))
            stage_b(b)
        for fn in pending_norm:
            fn()
        pending_norm.clear()
        if pace.get("forced") is not None:
            pace.pop("forced")()
        while filler:
            _c, fn, follow, _d = filler.popleft()
            fn()
            if follow is not None:
                follow()

    # Exp and Ln both live in the natural_log_exp_and_others ACT table set;
    # hide the single-function sets so the chooser can't thrash between them
    # (each ACT_TABLE_LOAD swap costs ~2.7us and stalls the exp stream).
    import concourse.bacc as bacc_mod

    orig_gat = bacc_mod.get_activation_tables

    def _combined_tables(arch):
        # keep positions intact (act_func_set_id indexes this list); just
        # empty the sets we don't want so the chooser can't pick them
        tabs = dict(orig_gat(arch))
        for bad in ("exp_and_others", "natural_log", "exp_and_friends"):
            if bad in tabs:
                tabs[bad] = type(tabs[bad])()
        return tabs

    if os.environ.get("ATTN_KERNEL_TABLES", "1") == "1":
        bacc_mod.get_activation_tables = _combined_tables
    try:
        nc.compile()
    finally:
        bacc_mod.get_activation_tables = orig_gat
    return nc


_PROGRAM = None
_LAST_EXEC_NS = None
_LAST_RESULT = None


def _get_program():
    global _PROGRAM
    if _PROGRAM is None:
        _PROGRAM = _build_program()
    return _PROGRAM


def kernel(x, mask, w_qkv, w_out):
    x = np.asarray(x)
    mask = np.asarray(mask)
    w_qkv = np.asarray(w_qkv)
    w_out = np.asarray(w_out)
    if not mask.all():
        return _masked_fallback(x, mask, w_qkv, w_out)

    from concourse.bass_utils import run_bass_kernel_spmd

    xT = np.ascontiguousarray(x.reshape(T, D).T).astype(np.float16)
    w4 = w_qkv.reshape(D, 3, NH, HD)
    ones = np.ones((128, 64), dtype=np.float16)
    ident = np.eye(128, dtype=np.float16)
    in_maps = []
    for c in range(NCORES):
        hsel = [2 * c, 2 * c + 1]
        wc = w4[:, :, hsel, :].reshape(D, 384).astype(np.float16)
        woc = w_out[2 * c * HD : (2 * c + 2) * HD, :].astype(np.float16)
        in_maps.append(
            {"xT": xT, "wqkv": wc, "wout": woc, "ones": ones, "ident": ident}
        )

    nc = _get_program()
    trace = os.environ.get("BASS_KERNEL_TRACE") == "1"
    res = run_bass_kernel_spmd(nc, in_maps, list(range(NCORES)), trace=trace)
    global _LAST_EXEC_NS, _LAST_RESULT
    _LAST_RESULT = res
    _LAST_EXEC_NS = getattr(res, "exec_time_ns", None)
    y = res.results[0]["y"].astype(np.float64)
    for c in range(1, NCORES):
        y += res.results[c]["y"]
    return y.astype(np.float32).reshape(B, L, D)


def _masked_fallback(x, mask, w_qkv, w_out):
    """Reference path for non-all-true masks (never hit for the spec inputs)."""
    b, l, d = x.shape
    scale = HD ** -0.5
    qkv = x.reshape(b * l, d) @ w_qkv
    qkv = qkv.reshape(b, l, 3, NH, HD).transpose(2, 0, 3, 1, 4)
    q, k, v = qkv[0], qkv[1], qkv[2]
    attn = np.einsum("bhnd,bhmd->bhnm", q, k) * scale
    attn = np.where(mask[:, None, :, :], attn, -np.inf)
    attn = attn - attn.max(axis=-1, keepdims=True)
    np.exp(attn, out=attn)
    attn /= attn.sum(axis=-1, keepdims=True)
    out = np.einsum("bhnm,bhmd->bhnd", attn, v)
    out = out.transpose(0, 2, 1, 3).reshape(b, l, d)
    return (out @ w_out).astype(np.float32)
